# revision 35
# baseline (speedup 1.0000x reference)
"""Minibatch discrimination kernel for 8 Trainium2 NeuronCores.

Reference computation:
    m = (x @ T.reshape(512, 128*32)).reshape(B=128, O=128, K=32)
    norm[i,j,o] = sum_k |m[i,o,k] - m[j,o,k]|
    o_b[j,o]    = sum_i exp(-norm[i,j,o]) - 1
    out         = concat([x, o_b], axis=1)            # [128, 640]

Distribution: shard the output-feature dim O=128 across the 8 cores (16
o's per core); no collectives.  Each core runs the GEMM for its T-slice
and the BxB pairwise exp-sum for its o-slice.

Per-core dataflow (tiles are [partition, free]):
  - GEMM -> M per o-group g as [(4o x 32k)=128 partitions, i=128] in
    PSUM; evicted to bf16 m_bf plus an exact f32 upcast m32 (the
    per-partition scalar / activation bias source).
  - norm is symmetric, so quad q (4 j's) only computes columns
    i >= 4q: free dim shrinks 128 -> 4 across quads, halving the
    elementwise volume.  The missing i < 4q part of o_b comes back via
    per-quad column sums (see below).
  - |d| tiles, one fused op per (j-region, o-group):
      DVE / GpSimd: tensor_scalar(subtract, max, 0) = relu(m_i - m_j)
        (weight-2 selector + P-correction; abs is not in the DVE/Pool
        hw ISA),
      ScalarE: activation Abs(-m + bias m32[:,j]) = |d| directly
        (weight-1 selector, no correction).
    A static plan balances the three engines' busy time.
  - k-reduction on the TensorEngine: per quad one seed matmul deposits
    -P_S[i,o] (P_S = sum over the RELU-produced groups of that row's
    region, host-precomputed per quad in c1) and per tile a selector
    matmul accumulates into the [(4j x 32(16o+16pad)), i] PSUM tile.
    Region jj=0 packs its tiles as fp8 pairs consumed by DoubleRow
    matmuls (0.5 cyc/row; hw requires dst partition base 0, so only
    this region qualifies).
  - One ScalarE Exp per quad with bias +P_S[j,o] (rides the bq table)
    -> E in SBUF bf16, two quads per E tile (odd quads' surplus PSUM
    columns are pre-seeded with +1e4 by a tiny PE matmul so they exp
    to exactly 0).  Row sums via one DVE free-axis reduce per quad
    PAIR into rs[:, q:q+2]; column sums via one PE matmul per quad
    accumulating sel4^T @ E_q[:, 4:] into ACC[16 o, j] over columns
    j >= 4q+4 only, so ACC[o,j] ends as sum_{i<4q_j} E[i,j] (prefix by
    construction).
  - Host combines o_b[j,o] = rs + ACC - 1 and concats with x.

Scheduling: every a/E tile is a unique SBUF allocation (pool-slot
reuse would attach WAR waits; in the cost model even a satisfied wait
costs ~30ns per instruction), exps lag their quads by ELAG and
rowsum/colsum by LAG more (tapered near the end), GEMM inputs arrive
as one fused tt_g0+x DMA, and rs ships in chunks as quads complete.
"""

import numpy as np
import ml_dtypes

import concourse.bacc as bacc
import concourse.tile as tile
import concourse.mybir as mybir
from concourse.bass_utils import run_bass_kernel_spmd

BF16 = ml_dtypes.bfloat16
FP8 = ml_dtypes.float8_e4m3

B = 128          # batch
IN_F = 512       # in_features
OUT_F = 128      # out_features
KD = 32          # kernel dim
N_CORES = 8
O_PER_CORE = OUT_F // N_CORES        # 16
N_GRP = O_PER_CORE * KD // 128       # 4 o-groups of (4 o x 32 k) partitions
O_PER_GRP = 128 // KD                # 4
JQ = 4                               # j's per PSUM tile / exp instruction
N_QUAD = B // JQ                     # 32
MW = 32                              # matmul M width per j (16 real + 16 zero)
LAG = 6                              # quads between exp and rowsum/colsum
ELAG = 6                             # quads between norm-psum and exp


def _plan():
    """Static engine plan for the 512 (q, jj, g) |d| tiles.

    Greedy makespan balance using the TimelineSim engine-busy costs
    (f = 128-4q): DVE 60.4+0.260f, ScalarE 185+0.833f, GpSimd
    95+1.389f.  Fixed loads: DVE carries evictions/upcasts/rowsums,
    ScalarE the exps.  Tiles within a quad are interchangeable, so the
    per-quad engine multiset is then packed into regions with the
    non-DVE tiles concentrated at region 0 (the only DoubleRow-legal
    dst), paired for fp8.

    Returns regions[q][jj] = list of 4 labels in {'D','A','P'} and
    pair8[q] = (pair0_is_fp8, pair1_is_fp8) for region 0.
    """
    load = {
        "D": 254 + 4 * 258 + 127 + sum(95 + 0.52 * (128 - 4 * q)
                             for q in range(0, N_QUAD, 2)),
        "A": sum(185 + 0.833 * (128 - 4 * q) for q in range(N_QUAD)),
        "P": 3 * 310.0,
    }
    cost = {
        "D": lambda f: 60.4 + 0.260 * f,
        "A": lambda f: 185 + 0.833 * f,
        "P": lambda f: 95 + 1.389 * f,
    }
    counts = [{"D": 0, "A": 0, "P": 0} for _ in range(N_QUAD)]
    tiles = [(128 - 4 * q, q) for q in range(N_QUAD)
             for _ in range(JQ * N_GRP)]
    tiles.sort(key=lambda t: -t[0])
    for f, q in tiles:
        pick = min(cost, key=lambda e: load[e] + cost[e](f))
        load[pick] += cost[pick](f)
        counts[q][pick] += 1
    regions = []
    pair8 = []
    for q in range(N_QUAD):
        c = dict(counts[q])
        nond = ["P"] * c["P"] + ["A"] * c["A"]
        dd = ["D"] * c["D"]
        r0 = [(nond or dd).pop(0) for _ in range(N_GRP)]
        rest = nond + dd
        regs = [r0] + [[rest.pop(0) for _ in range(N_GRP)]
                       for _ in range(1, JQ)]
        pair8.append((r0[0] != "D" and r0[1] != "D",
                      r0[2] != "D" and r0[3] != "D"))
        regions.append(regs)
    return regions, pair8, load


_REG, _PAIR8, _LOAD = _plan()


def _build():
    f32, bf16 = mybir.dt.float32, mybir.dt.bfloat16
    fp8 = mybir.dt.float8e4
    A = mybir.AluOpType
    AF = mybir.ActivationFunctionType
    nc = bacc.Bacc("TRN2", target_bir_lowering=False, debug=False)

    tg_d = nc.dram_tensor("txg", [128, IN_F // 128, 256], bf16, kind="ExternalInput")
    tt_d = nc.dram_tensor("tt", [N_GRP - 1, 128, IN_F // 128, 128], bf16, kind="ExternalInput")
    s2b_d = nc.dram_tensor("s2b", [128, 2, N_GRP, MW], bf16, kind="ExternalInput")
    s8_d = nc.dram_tensor("s8", [128, 2, 4, 2, MW], fp8, kind="ExternalInput")
    sel4_d = nc.dram_tensor("sel4", [128, O_PER_CORE], bf16, kind="ExternalInput")
    id_d = nc.dram_tensor("idm", [128, 128], bf16, kind="ExternalInput")
    c1_d = nc.dram_tensor("c1", [B, N_QUAD, 128], bf16, kind="ExternalInput")
    bq_d = nc.dram_tensor("bq", [128, N_QUAD], f32, kind="ExternalInput")
    rs_d = nc.dram_tensor("rs", [128, N_QUAD], f32, kind="ExternalOutput")
    acc_d = nc.dram_tensor("accs", [O_PER_CORE, B - JQ], f32, kind="ExternalOutput")

    n_chunk = IN_F // 128  # 4 contraction chunks

    with tile.TileContext(nc) as tc:
        with (
            tc.tile_pool(name="singles", bufs=1) as singles,
            tc.tile_pool(name="psn", bufs=5, space="PSUM") as psn,
            tc.tile_pool(name="psa", bufs=1, space="PSUM") as psa,
        ):
            # --- warm the ACT exp/abs tables while DMAs run ---
            warm = singles.tile([1, 4], mybir.dt.float32, tag="warm")
            nc.vector.memset(warm[:], 0.0)
            nc.scalar.activation(
                out=warm[0:1, 0:1], in_=warm[0:1, 1:2],
                func=AF.Exp, bias=0.0, scale=-1.0,
            )
            nc.scalar.activation(
                out=warm[0:1, 2:3], in_=warm[0:1, 3:4],
                func=AF.Abs, bias=0.0, scale=-1.0,
            )

            # --- batched input DMAs, all on the SP queue ---
            # tt: dram [512, 512] -> sbuf [128, 4c, 512]
            tg_sb = singles.tile([128, n_chunk, 256], bf16, tag="tg")
            t_sb = singles.tile([128, N_GRP - 1, n_chunk, 128], bf16, tag="t")
            c1_sb = singles.tile([B, N_QUAD, 128], bf16, tag="c1")
            s2b_sb = singles.tile([128, 2, N_GRP, MW], bf16, tag="s2b")
            s8_sb = singles.tile([128, 2, 4, 2, MW], fp8, tag="s8")
            sel4_sb = singles.tile([128, O_PER_CORE], bf16, tag="sel4")
            id_sb = singles.tile([128, 128], bf16, tag="idm")
            bq_sb = singles.tile([128, N_QUAD], f32, tag="bq")
            nc.sync.dma_start(tg_sb[:], tg_d[:])
            nc.sync.dma_start(t_sb[:, 0, :, :], tt_d[0])
            nc.sync.dma_start(t_sb[:, 1, :, :], tt_d[1])
            nc.sync.dma_start(t_sb[:, 2, :, :], tt_d[2])
            nc.sync.dma_start(c1_sb[:, 0:8, :], c1_d[:, 0:8, :])
            nc.sync.dma_start(s2b_sb[:], s2b_d[:])
            nc.sync.dma_start(s8_sb[:], s8_d[:])
            nc.sync.dma_start(sel4_sb[:], sel4_d[:])
            nc.sync.dma_start(id_sb[:], id_d[:])
            nc.sync.dma_start(bq_sb[:], bq_d[:])
            nc.sync.dma_start(c1_sb[:, 8:20, :], c1_d[:, 8:20, :])
            nc.sync.dma_start(c1_sb[:, 20:32, :], c1_d[:, 20:32, :])

            # --- GEMM: M[g] = (T_g)^T x^T : [(4o,32k)=128, i=128] ---
            m_bf = []
            m32 = []
            gemm_pool_cm = tc.tile_pool(name="psg", bufs=2, space="PSUM")
            psg = gemm_pool_cm.__enter__()
            for g in range(N_GRP):
                pg = psg.tile([128, B], f32, tag="gemm")
                for c in range(n_chunk):
                    lhsT = (tg_sb[:, c, 0:128] if g == 0
                            else t_sb[:, g - 1, c, :])
                    nc.tensor.matmul(
                        pg[:],
                        lhsT,
                        tg_sb[:, c, 128:256],
                        start=(c == 0),
                        stop=(c == n_chunk - 1),
                    )
                mb = singles.tile([128, B], bf16, tag=f"mb{g}")
                if g in (1, 3):
                    nc.scalar.activation(             # PSUM -> SBUF, bf16
                        out=mb[:], in_=pg[:],
                        func=AF.Copy, bias=0.0, scale=1.0)
                else:
                    nc.vector.tensor_copy(mb[:], pg[:])
                m_bf.append(mb)
                mu = singles.tile([128, B], f32, tag=f"mu{g}")
                ueng = nc.vector if g < 2 else nc.gpsimd
                ueng.tensor_copy(mu[:], mb[:])        # exact f32 upcast
                m32.append(mu)
            gemm_pool_cm.__exit__(None, None, None)

            # --- pairwise: per j-quad, |d| tiles -> k-reduce -> exp ---
            rs_sb = singles.tile([128, N_QUAD], f32, tag="rs")
            lw_sb = singles.tile([128, 128], bf16, tag="lw")
            nc.vector.memset(lw_sb[:], 10000.0)
            acc_ps = psa.tile([O_PER_CORE, B], f32, tag="accp")
            pending = []

            def emit_tile(eng, dst, g, j, i0):
                if eng == "D":
                    nc.vector.tensor_scalar(
                        out=dst, in0=m_bf[g][:, i0:B],
                        scalar1=m32[g][:, j:j + 1], scalar2=0.0,
                        op0=A.subtract, op1=A.max,
                    )
                elif eng == "A":
                    nc.scalar.activation(
                        out=dst, in_=m_bf[g][:, i0:B],
                        func=AF.Abs,
                        bias=m32[g][:, j:j + 1], scale=-1.0,
                    )
                else:
                    nc.gpsimd.tensor_scalar(
                        out=dst, in0=m_bf[g][:, i0:B],
                        scalar1=m32[g][:, j:j + 1], scalar2=0.0,
                        op0=A.subtract, op1=A.max,
                    )

            exp_pending = []

            epairs = {}

            def emit_exp(q, pn_tile, f):
                slot = q % 2
                if slot == 0:
                    epairs[q] = singles.tile([128, 2, f], bf16, tag=f"e_{q}", name=f"e_{q}")
                e_pair = epairs[q - slot]
                fpad = 128 - 4 * (q - slot)
                nc.scalar.activation(
                    out=e_pair[:, slot, 0:fpad], in_=pn_tile[:, 0:fpad],
                    func=AF.Exp, bias=bq_sb[:, q:q + 1], scale=-1.0,
                )
                pending.append((q, e_pair, slot, f, fpad))
                lag = LAG if q < N_QUAD - 5 else 2
                while len(pending) > lag:
                    finish(*pending.pop(0))

            def finish(q, e_pair, slot, f, fpad):
                if slot == 1:
                    nc.vector.tensor_reduce(
                        out=rs_sb[:, q - 1:q + 1], in_=e_pair[:, :, 0:fpad],
                        axis=mybir.AxisListType.X, op=A.add,
                    )
                if q == 15:
                    nc.sync.dma_start(rs_d[:, 0:16], rs_sb[:, 0:16])
                elif q == 27:
                    nc.sync.dma_start(rs_d[:, 16:28], rs_sb[:, 16:28])
                if q < N_QUAD - 1:
                    nc.tensor.matmul(
                        acc_ps[:, 4 * q + JQ:B], sel4_sb[:],
                        e_pair[:, slot, JQ:f],
                        start=(q == 0), stop=(q == N_QUAD - 2),
                        skip_group_check=True,
                    )
                if q == N_QUAD - 2:
                    acc_sb = singles.tile([O_PER_CORE, B - JQ], f32,
                                          tag="acc_sb")
                    nc.vector.tensor_copy(acc_sb[:], acc_ps[:, JQ:B])
                    nc.gpsimd.dma_start(acc_d[:], acc_sb[:])

            for q in range(N_QUAD):
                i0 = 4 * q
                f = 128 - i0
                pn = psn.tile([128, B], f32, tag="norm")
                # seed: pn[row, i] = -P_S(row)[i, o(row)] for all 4 regions
                nc.tensor.matmul(
                    pn[:, 0:f], c1_sb[:, q, :], id_sb[:, i0:B],
                    start=True, stop=False, skip_group_check=True,
                )
                for jj in range(JQ):
                    j = JQ * q + jj
                    labels = _REG[q][jj]
                    reg = pn[MW * jj:MW * (jj + 1), 0:f]
                    mms = []
                    if jj == 0:
                        for pr in range(2):
                            gs = (2 * pr, 2 * pr + 1)
                            if _PAIR8[q][pr]:
                                f16 = (f + 15) // 16 * 16
                                a8 = singles.tile([128, 2, f16], fp8,
                                                  tag=f"a8_{q}_{pr}")
                                for t, g in enumerate(gs):
                                    emit_tile(labels[g], a8[:, t, 0:f], g, j, i0)
                                # selector variant by (slot0, slot1) weights
                                v = ((labels[gs[0]] == "P") * 2
                                     + (labels[gs[1]] == "P") * 1)
                                mms.append(("dr", pr, v, a8))
                            else:
                                for g in gs:
                                    ad = singles.tile([128, f], bf16,
                                                      tag=f"a_{q}_{jj}_{g}")
                                    emit_tile(labels[g], ad[:, 0:f], g, j, i0)
                                    mms.append(("b", g, labels[g], ad))
                    else:
                        for g in range(N_GRP):
                            ad = singles.tile([128, f], bf16,
                                              tag=f"a_{q}_{jj}_{g}")
                            emit_tile(labels[g], ad[:, 0:f], g, j, i0)
                            mms.append(("b", g, labels[g], ad))
                    for idx, mm in enumerate(mms):
                        stop = idx == len(mms) - 1
                        if mm[0] == "dr":
                            _, pr, v, a8 = mm
                            nc.tensor.matmul(
                                reg, s8_sb[:, pr, v, :, :], a8[:, :, 0:f],
                                start=False, stop=stop,
                                perf_mode=mybir.MatmulPerfMode.DoubleRow,
                                tile_position=(0, MW * jj),
                                skip_group_check=True,
                            )
                        else:
                            _, g, lab, ad = mm
                            w = 0 if lab != "A" else 1   # 0: weight 2, 1: weight 1
                            nc.tensor.matmul(
                                reg, s2b_sb[:, w, g, :], ad[:, 0:f],
                                start=False, stop=stop,
                                tile_position=(0, MW * jj),
                                skip_group_check=True,
                            )

                if q % 2 == 1:
                    # pad pn cols f..f+4 with +1e4 so they exp to 0
                    nc.tensor.matmul(
                        pn[:, f:f + 4], lw_sb[:], id_sb[:, 0:4],
                        start=True, stop=True, skip_group_check=True,
                    )
                exp_pending.append((q, pn, f))
                elag = ELAG if q < N_QUAD - 4 else 2
                while len(exp_pending) > elag:
                    emit_exp(*exp_pending.pop(0))

            while exp_pending:
                emit_exp(*exp_pending.pop(0))
            while pending:
                finish(*pending.pop(0))

            # --- ship the last rs columns ---
            nc.sync.dma_start(rs_d[:, 28:N_QUAD], rs_sb[:, 28:N_QUAD])

    nc.compile()
    return nc


_NC = None


def kernel(x: np.ndarray, T: np.ndarray) -> np.ndarray:
    global _NC
    if _NC is None:
        _NC = _build()
    nc = _NC

    x = np.ascontiguousarray(x, dtype=np.float32)
    T = np.ascontiguousarray(T, dtype=np.float32)

    xt = x.T.reshape(IN_F // 128, 128, B).transpose(1, 0, 2).astype(BF16)

    # selectors: col g*4 + o_loc, weight 2 (relu tiles) or 1 (abs tiles)
    s2b = np.zeros((128, 2, N_GRP, MW), dtype=BF16)
    for p in range(128):
        o_loc = p // KD
        for g in range(N_GRP):
            s2b[p, 0, g, g * O_PER_GRP + o_loc] = 2
            s2b[p, 1, g, g * O_PER_GRP + o_loc] = 1
    # fp8 DoubleRow selector: pair pr covers groups (2pr, 2pr+1); variant
    # v encodes (slot0_weight==2)*2 + (slot1_weight==2)*1
    s8 = np.zeros((128, 2, 4, 2, MW), dtype=FP8)
    for p in range(128):
        o_loc = p // KD
        for pr in range(2):
            for v in range(4):
                w0 = 2 if v & 2 else 1
                w1 = 2 if v & 1 else 1
                g0, g1 = 2 * pr, 2 * pr + 1
                s8[p, pr, v, 0, g0 * O_PER_GRP + o_loc] = w0
                s8[p, pr, v, 1, g1 * O_PER_GRP + o_loc] = w1
    # colsum selector: partition (jj, c) -> column c (c < 16)
    sel4 = np.zeros((128, O_PER_CORE), dtype=BF16)
    for jj in range(JQ):
        for c in range(O_PER_CORE):
            sel4[MW * jj + c, c] = 1
    ident = np.eye(128, dtype=BF16)

    # host-side P[i, o] = sum_k m[i, o, k]; each o belongs to exactly one
    # group g = (o mod 16) // 4, so the relu correction for row (jj, o)
    # is P[i, o] masked by whether that region's group-g tile is
    # relu-produced (label != 'A').  Only consistency with the device's
    # bf16 m matters (the +P/-P copies cancel exactly on the diagonal).
    m_host = (x @ T.reshape(IN_F, OUT_F * KD)).reshape(B, OUT_F, KD)
    P_all = m_host.sum(axis=-1)                                 # [B, 128]

    in_maps = []
    for core in range(N_CORES):
        t_slice = T[:, core * O_PER_CORE:(core + 1) * O_PER_CORE, :]
        # [g][p(in_f within chunk), c, (4o,32k) col]
        tw = t_slice.reshape(IN_F // 128, 128, N_GRP, 128)
        ttg = tw.transpose(2, 1, 0, 3).astype(BF16)       # [4, 128, 4, 128]
        txg = np.concatenate([ttg[0], xt], axis=2)        # [128, 4, 256]
        txg = np.ascontiguousarray(txg)
        tt = np.ascontiguousarray(ttg[1:])
        P = P_all[:, core * O_PER_CORE:(core + 1) * O_PER_CORE]  # [B, 16]
        Pb = P.astype(BF16)                   # bf16-quantized, used as-is
        Pf = Pb.astype(np.float32)
        # c1[i, q, row(jj,c)] = -bf16(P_S); bq[row, q] = -f32(bf16(P_S))[j]
        c1 = np.zeros((B, N_QUAD, 128), dtype=BF16)
        bq = np.zeros((128, N_QUAD), dtype=np.float32)
        for q in range(N_QUAD):
            for jj in range(JQ):
                labels = _REG[q][jj]
                mask = np.array([labels[c // O_PER_GRP] != "A"
                                 for c in range(O_PER_CORE)])
                c1[:, q, MW * jj:MW * jj + O_PER_CORE] = \
                    np.where(mask[None, :], -Pf, 0.0).astype(BF16)
                bq[MW * jj:MW * jj + O_PER_CORE, q] = \
                    np.where(mask, -Pf[JQ * q + jj, :], 0.0)
        in_maps.append({"tt": tt, "txg": txg, "s2b": s2b, "s8": s8,
                        "sel4": sel4, "idm": ident, "c1": c1, "bq": bq})

    res = run_bass_kernel_spmd(nc, in_maps, core_ids=list(range(N_CORES)))

    ob_full = np.empty((B, OUT_F), dtype=np.float32)
    for c, r in enumerate(res.results):
        rs = r["rs"]                                            # [128, 32]
        ac = r["accs"]                                          # [16, 124]
        row = rs.reshape(JQ, MW, N_QUAD)[:, :O_PER_CORE, :]     # [jj, r, q]
        ob = row.transpose(2, 0, 1).reshape(B, O_PER_CORE)      # [j, r]
        ob[JQ:, :] += ac.T                                      # j >= 4
        ob_full[:, c * O_PER_CORE:(c + 1) * O_PER_CORE] = ob
    out = np.concatenate([x, ob_full - 1.0], axis=1).astype(np.float32)
    return out


if __name__ == "__main__":
    print("plan loads (ns):", {k: round(v) for k, v in _LOAD.items()})
    n8 = sum(p[0] + p[1] for p in _PAIR8)
    print(f"fp8 DR pairs: {n8}/64")


# revision 37
# speedup vs baseline: 1.0066x; 1.0066x over previous
"""Minibatch discrimination kernel for 8 Trainium2 NeuronCores.

Reference computation:
    m = (x @ T.reshape(512, 128*32)).reshape(B=128, O=128, K=32)
    norm[i,j,o] = sum_k |m[i,o,k] - m[j,o,k]|
    o_b[j,o]    = sum_i exp(-norm[i,j,o]) - 1
    out         = concat([x, o_b], axis=1)            # [128, 640]

Distribution: shard the output-feature dim O=128 across the 8 cores (16
o's per core); no collectives.  Each core runs the GEMM for its T-slice
and the BxB pairwise exp-sum for its o-slice.

Per-core dataflow (tiles are [partition, free]):
  - GEMM -> M per o-group g as [(4o x 32k)=128 partitions, i=128] in
    PSUM; evicted to bf16 m_bf plus an exact f32 upcast m32 (the
    per-partition scalar / activation bias source).
  - norm is symmetric, so quad q (4 j's) only computes columns
    i >= 4q: free dim shrinks 128 -> 4 across quads, halving the
    elementwise volume.  The missing i < 4q part of o_b comes back via
    per-quad column sums (see below).
  - |d| tiles, one fused op per (j-region, o-group):
      DVE / GpSimd: tensor_scalar(subtract, max, 0) = relu(m_i - m_j)
        (weight-2 selector + P-correction; abs is not in the DVE/Pool
        hw ISA),
      ScalarE: activation Abs(-m + bias m32[:,j]) = |d| directly
        (weight-1 selector, no correction).
    A static plan balances the three engines' busy time.
  - k-reduction on the TensorEngine: per quad one seed matmul deposits
    -P_S[i,o] (P_S = sum over the RELU-produced groups of that row's
    region, host-precomputed per quad in c1) and per tile a selector
    matmul accumulates into the [(4j x 32(16o+16pad)), i] PSUM tile.
    Region jj=0 packs its tiles as fp8 pairs consumed by DoubleRow
    matmuls (0.5 cyc/row; hw requires dst partition base 0, so only
    this region qualifies).
  - One ScalarE Exp per quad with bias +P_S[j,o] (rides the bq table)
    -> E in SBUF bf16, two quads per E tile (odd quads' surplus PSUM
    columns are pre-seeded with +1e4 by a tiny PE matmul so they exp
    to exactly 0).  Row sums via one DVE free-axis reduce per quad
    PAIR into rs[:, q:q+2]; column sums via one PE matmul per quad
    accumulating sel4^T @ E_q[:, 4:] into ACC[16 o, j] over columns
    j >= 4q+4 only, so ACC[o,j] ends as sum_{i<4q_j} E[i,j] (prefix by
    construction).
  - Host combines o_b[j,o] = rs + ACC - 1 and concats with x.

Scheduling: every a/E tile is a unique SBUF allocation (pool-slot
reuse would attach WAR waits; in the cost model even a satisfied wait
costs ~30ns per instruction), exps lag their quads by ELAG and
rowsum/colsum by LAG more (tapered near the end), GEMM inputs arrive
as one fused tt_g0+x DMA, and rs ships in chunks as quads complete.
"""

import numpy as np
import ml_dtypes

import concourse.bacc as bacc
import concourse.tile as tile
import concourse.mybir as mybir
from concourse.bass_utils import run_bass_kernel_spmd

BF16 = ml_dtypes.bfloat16
FP8 = ml_dtypes.float8_e4m3

B = 128          # batch
IN_F = 512       # in_features
OUT_F = 128      # out_features
KD = 32          # kernel dim
N_CORES = 8
O_PER_CORE = OUT_F // N_CORES        # 16
N_GRP = O_PER_CORE * KD // 128       # 4 o-groups of (4 o x 32 k) partitions
O_PER_GRP = 128 // KD                # 4
JQ = 4                               # j's per PSUM tile / exp instruction
N_QUAD = B // JQ                     # 32
MW = 32                              # matmul M width per j (16 real + 16 zero)
LAG = 6                              # quads between exp and rowsum/colsum
ELAG = 6                             # quads between norm-psum and exp


def _plan():
    """Static engine plan for the 512 (q, jj, g) |d| tiles.

    Greedy makespan balance using the TimelineSim engine-busy costs
    (f = 128-4q): DVE 60.4+0.260f, ScalarE 185+0.833f, GpSimd
    95+1.389f.  Fixed loads: DVE carries evictions/upcasts/rowsums,
    ScalarE the exps.  Tiles within a quad are interchangeable, so the
    per-quad engine multiset is then packed into regions with the
    non-DVE tiles concentrated at region 0 (the only DoubleRow-legal
    dst), paired for fp8.

    Returns regions[q][jj] = list of 4 labels in {'D','A','P'} and
    pair8[q] = (pair0_is_fp8, pair1_is_fp8) for region 0.
    """
    load = {
        "D": 254 + 4 * 258 + 127 + sum(95 + 0.52 * (128 - 4 * q)
                             for q in range(0, N_QUAD, 2)),
        "A": sum(185 + 0.833 * (128 - 4 * q) for q in range(N_QUAD)),
        "P": 3 * 310.0,
    }
    cost = {
        "D": lambda f: 60.4 + 0.260 * f,
        "A": lambda f: 185 + 0.833 * f,
        "P": lambda f: 95 + 1.389 * f,
    }
    counts = [{"D": 0, "A": 0, "P": 0} for _ in range(N_QUAD)]
    tiles = [(128 - 4 * q, q) for q in range(N_QUAD)
             for _ in range(JQ * N_GRP)]
    tiles.sort(key=lambda t: -t[0])
    for f, q in tiles:
        pick = min(cost, key=lambda e: load[e] + cost[e](f))
        load[pick] += cost[pick](f)
        counts[q][pick] += 1
    regions = []
    pair8 = []
    for q in range(N_QUAD):
        c = dict(counts[q])
        nond = ["P"] * c["P"] + ["A"] * c["A"]
        dd = ["D"] * c["D"]
        r0 = [(nond or dd).pop(0) for _ in range(N_GRP)]
        rest = nond + dd
        regs = [r0] + [[rest.pop(0) for _ in range(N_GRP)]
                       for _ in range(1, JQ)]
        pair8.append((r0[0] != "D" and r0[1] != "D",
                      r0[2] != "D" and r0[3] != "D"))
        regions.append(regs)
    return regions, pair8, load


_REG, _PAIR8, _LOAD = _plan()


def _build():
    f32, bf16 = mybir.dt.float32, mybir.dt.bfloat16
    fp8 = mybir.dt.float8e4
    A = mybir.AluOpType
    AF = mybir.ActivationFunctionType
    nc = bacc.Bacc("TRN2", target_bir_lowering=False, debug=False)

    tg_d = nc.dram_tensor("txg", [128, IN_F // 128, 256], bf16, kind="ExternalInput")
    tt_d = nc.dram_tensor("tt", [N_GRP - 1, 128, IN_F // 128, 128], bf16, kind="ExternalInput")
    s2b_d = nc.dram_tensor("s2b", [128, 2, N_GRP, MW], bf16, kind="ExternalInput")
    s8_d = nc.dram_tensor("s8", [128, 2, 4, 2, MW], fp8, kind="ExternalInput")
    sel4_d = nc.dram_tensor("sel4", [128, O_PER_CORE], bf16, kind="ExternalInput")
    id_d = nc.dram_tensor("idm", [128, 128], bf16, kind="ExternalInput")
    c1_d = nc.dram_tensor("c1", [B, N_QUAD, 128], bf16, kind="ExternalInput")
    bq_d = nc.dram_tensor("bq", [128, N_QUAD], f32, kind="ExternalInput")
    rs_d = nc.dram_tensor("rs", [128, N_QUAD], f32, kind="ExternalOutput")
    acc_d = nc.dram_tensor("accs", [O_PER_CORE, B - JQ], f32, kind="ExternalOutput")

    n_chunk = IN_F // 128  # 4 contraction chunks

    with tile.TileContext(nc) as tc:
        with (
            tc.tile_pool(name="singles", bufs=1) as singles,
            tc.tile_pool(name="psn", bufs=5, space="PSUM") as psn,
            tc.tile_pool(name="psa", bufs=1, space="PSUM") as psa,
        ):
            # --- warm the ACT exp/abs tables while DMAs run ---
            warm = singles.tile([1, 4], mybir.dt.float32, tag="warm")
            nc.vector.memset(warm[:], 0.0)
            nc.scalar.activation(
                out=warm[0:1, 0:1], in_=warm[0:1, 1:2],
                func=AF.Exp, bias=0.0, scale=-1.0,
            )
            nc.scalar.activation(
                out=warm[0:1, 2:3], in_=warm[0:1, 3:4],
                func=AF.Abs, bias=0.0, scale=-1.0,
            )

            # --- batched input DMAs, all on the SP queue ---
            # tt: dram [512, 512] -> sbuf [128, 4c, 512]
            tg_sb = singles.tile([128, n_chunk, 256], bf16, tag="tg")
            t_sb = singles.tile([128, N_GRP - 1, n_chunk, 128], bf16, tag="t")
            c1_sb = singles.tile([B, N_QUAD, 128], bf16, tag="c1")
            s2b_sb = singles.tile([128, 2, N_GRP, MW], bf16, tag="s2b")
            s8_sb = singles.tile([128, 2, 4, 2, MW], fp8, tag="s8")
            sel4_sb = singles.tile([128, O_PER_CORE], bf16, tag="sel4")
            id_sb = singles.tile([128, 128], bf16, tag="idm")
            bq_sb = singles.tile([128, N_QUAD], f32, tag="bq")
            nc.sync.dma_start(tg_sb[:, 0:2, :], tg_d[:, 0:2, :])
            nc.sync.dma_start(tg_sb[:, 2:4, :], tg_d[:, 2:4, :])
            nc.sync.dma_start(t_sb[:, 0, :, :], tt_d[0])
            nc.sync.dma_start(t_sb[:, 1, :, :], tt_d[1])
            nc.sync.dma_start(t_sb[:, 2, :, :], tt_d[2])
            nc.sync.dma_start(c1_sb[:, 0:8, :], c1_d[:, 0:8, :])
            nc.sync.dma_start(s2b_sb[:], s2b_d[:])
            nc.sync.dma_start(s8_sb[:], s8_d[:])
            nc.sync.dma_start(sel4_sb[:], sel4_d[:])
            nc.sync.dma_start(id_sb[:], id_d[:])
            nc.sync.dma_start(bq_sb[:], bq_d[:])
            nc.sync.dma_start(c1_sb[:, 8:20, :], c1_d[:, 8:20, :])
            nc.sync.dma_start(c1_sb[:, 20:32, :], c1_d[:, 20:32, :])

            # --- GEMM: M[g] = (T_g)^T x^T : [(4o,32k)=128, i=128] ---
            m_bf = []
            m32 = []
            gemm_pool_cm = tc.tile_pool(name="psg", bufs=2, space="PSUM")
            psg = gemm_pool_cm.__enter__()
            for g in range(N_GRP):
                pg = psg.tile([128, B], f32, tag="gemm")
                for c in range(n_chunk):
                    lhsT = (tg_sb[:, c, 0:128] if g == 0
                            else t_sb[:, g - 1, c, :])
                    nc.tensor.matmul(
                        pg[:],
                        lhsT,
                        tg_sb[:, c, 128:256],
                        start=(c == 0),
                        stop=(c == n_chunk - 1),
                    )
                mb = singles.tile([128, B], bf16, tag=f"mb{g}")
                if g in (1, 3):
                    nc.scalar.activation(             # PSUM -> SBUF, bf16
                        out=mb[:], in_=pg[:],
                        func=AF.Copy, bias=0.0, scale=1.0)
                else:
                    nc.vector.tensor_copy(mb[:], pg[:])
                m_bf.append(mb)
                mu = singles.tile([128, B], f32, tag=f"mu{g}")
                nc.vector.tensor_copy(mu[:], mb[:])   # exact f32 upcast
                m32.append(mu)
            gemm_pool_cm.__exit__(None, None, None)

            # --- pairwise: per j-quad, |d| tiles -> k-reduce -> exp ---
            rs_sb = singles.tile([128, N_QUAD], f32, tag="rs")
            lw_sb = singles.tile([128, 128], bf16, tag="lw")
            nc.vector.memset(lw_sb[:], 10000.0)
            acc_ps = psa.tile([O_PER_CORE, B], f32, tag="accp")
            pending = []

            def emit_tile(eng, dst, g, j, i0):
                if eng == "D":
                    nc.vector.tensor_scalar(
                        out=dst, in0=m_bf[g][:, i0:B],
                        scalar1=m32[g][:, j:j + 1], scalar2=0.0,
                        op0=A.subtract, op1=A.max,
                    )
                elif eng == "A":
                    nc.scalar.activation(
                        out=dst, in_=m_bf[g][:, i0:B],
                        func=AF.Abs,
                        bias=m32[g][:, j:j + 1], scale=-1.0,
                    )
                else:
                    nc.gpsimd.tensor_scalar(
                        out=dst, in0=m_bf[g][:, i0:B],
                        scalar1=m32[g][:, j:j + 1], scalar2=0.0,
                        op0=A.subtract, op1=A.max,
                    )

            exp_pending = []

            epairs = {}

            def emit_exp(q, pn_tile, f):
                slot = q % 2
                if slot == 0:
                    epairs[q] = singles.tile([128, 2, f], bf16, tag=f"e_{q}", name=f"e_{q}")
                e_pair = epairs[q - slot]
                fpad = 128 - 4 * (q - slot)
                nc.scalar.activation(
                    out=e_pair[:, slot, 0:fpad], in_=pn_tile[:, 0:fpad],
                    func=AF.Exp, bias=bq_sb[:, q:q + 1], scale=-1.0,
                )
                pending.append((q, e_pair, slot, f, fpad))
                lag = LAG if q < N_QUAD - 5 else 2
                while len(pending) > lag:
                    finish(*pending.pop(0))

            def finish(q, e_pair, slot, f, fpad):
                if slot == 1:
                    nc.vector.tensor_reduce(
                        out=rs_sb[:, q - 1:q + 1], in_=e_pair[:, :, 0:fpad],
                        axis=mybir.AxisListType.X, op=A.add,
                    )
                if q == 15:
                    nc.sync.dma_start(rs_d[:, 0:16], rs_sb[:, 0:16])
                elif q == 27:
                    nc.sync.dma_start(rs_d[:, 16:28], rs_sb[:, 16:28])
                if q < N_QUAD - 1:
                    nc.tensor.matmul(
                        acc_ps[:, 4 * q + JQ:B], sel4_sb[:],
                        e_pair[:, slot, JQ:f],
                        start=(q == 0), stop=(q == N_QUAD - 2),
                        skip_group_check=True,
                    )
                if q == N_QUAD - 2:
                    acc_sb = singles.tile([O_PER_CORE, B - JQ], f32,
                                          tag="acc_sb")
                    nc.vector.tensor_copy(acc_sb[:], acc_ps[:, JQ:B])
                    nc.gpsimd.dma_start(acc_d[:], acc_sb[:])

            for q in range(N_QUAD):
                i0 = 4 * q
                f = 128 - i0
                pn = psn.tile([128, B], f32, tag="norm")
                # seed: pn[row, i] = -P_S(row)[i, o(row)] for all 4 regions
                nc.tensor.matmul(
                    pn[:, 0:f], c1_sb[:, q, :], id_sb[:, i0:B],
                    start=True, stop=False, skip_group_check=True,
                )
                for jj in range(JQ):
                    j = JQ * q + jj
                    labels = _REG[q][jj]
                    reg = pn[MW * jj:MW * (jj + 1), 0:f]
                    mms = []
                    if jj == 0:
                        for pr in range(2):
                            gs = (2 * pr, 2 * pr + 1)
                            if _PAIR8[q][pr]:
                                f16 = (f + 15) // 16 * 16
                                a8 = singles.tile([128, 2, f16], fp8,
                                                  tag=f"a8_{q}_{pr}")
                                for t, g in enumerate(gs):
                                    emit_tile(labels[g], a8[:, t, 0:f], g, j, i0)
                                # selector variant by (slot0, slot1) weights
                                v = ((labels[gs[0]] == "P") * 2
                                     + (labels[gs[1]] == "P") * 1)
                                mms.append(("dr", pr, v, a8))
                            else:
                                for g in gs:
                                    ad = singles.tile([128, f], bf16,
                                                      tag=f"a_{q}_{jj}_{g}")
                                    emit_tile(labels[g], ad[:, 0:f], g, j, i0)
                                    mms.append(("b", g, labels[g], ad))
                    else:
                        for g in range(N_GRP):
                            ad = singles.tile([128, f], bf16,
                                              tag=f"a_{q}_{jj}_{g}")
                            emit_tile(labels[g], ad[:, 0:f], g, j, i0)
                            mms.append(("b", g, labels[g], ad))
                    for idx, mm in enumerate(mms):
                        stop = idx == len(mms) - 1
                        if mm[0] == "dr":
                            _, pr, v, a8 = mm
                            nc.tensor.matmul(
                                reg, s8_sb[:, pr, v, :, :], a8[:, :, 0:f],
                                start=False, stop=stop,
                                perf_mode=mybir.MatmulPerfMode.DoubleRow,
                                tile_position=(0, MW * jj),
                                skip_group_check=True,
                            )
                        else:
                            _, g, lab, ad = mm
                            w = 0 if lab != "A" else 1   # 0: weight 2, 1: weight 1
                            nc.tensor.matmul(
                                reg, s2b_sb[:, w, g, :], ad[:, 0:f],
                                start=False, stop=stop,
                                tile_position=(0, MW * jj),
                                skip_group_check=True,
                            )

                if q % 2 == 1:
                    # pad pn cols f..f+4 with +1e4 so they exp to 0
                    nc.tensor.matmul(
                        pn[:, f:f + 4], lw_sb[:], id_sb[:, 0:4],
                        start=True, stop=True, skip_group_check=True,
                    )
                exp_pending.append((q, pn, f))
                elag = ELAG if q < N_QUAD - 4 else 2
                while len(exp_pending) > elag:
                    emit_exp(*exp_pending.pop(0))

            while exp_pending:
                emit_exp(*exp_pending.pop(0))
            while pending:
                finish(*pending.pop(0))

            # --- ship the last rs columns ---
            nc.sync.dma_start(rs_d[:, 28:N_QUAD], rs_sb[:, 28:N_QUAD])

    nc.compile()
    return nc


_NC = None


def kernel(x: np.ndarray, T: np.ndarray) -> np.ndarray:
    global _NC
    if _NC is None:
        _NC = _build()
    nc = _NC

    x = np.ascontiguousarray(x, dtype=np.float32)
    T = np.ascontiguousarray(T, dtype=np.float32)

    xt = x.T.reshape(IN_F // 128, 128, B).transpose(1, 0, 2).astype(BF16)

    # selectors: col g*4 + o_loc, weight 2 (relu tiles) or 1 (abs tiles)
    s2b = np.zeros((128, 2, N_GRP, MW), dtype=BF16)
    for p in range(128):
        o_loc = p // KD
        for g in range(N_GRP):
            s2b[p, 0, g, g * O_PER_GRP + o_loc] = 2
            s2b[p, 1, g, g * O_PER_GRP + o_loc] = 1
    # fp8 DoubleRow selector: pair pr covers groups (2pr, 2pr+1); variant
    # v encodes (slot0_weight==2)*2 + (slot1_weight==2)*1
    s8 = np.zeros((128, 2, 4, 2, MW), dtype=FP8)
    for p in range(128):
        o_loc = p // KD
        for pr in range(2):
            for v in range(4):
                w0 = 2 if v & 2 else 1
                w1 = 2 if v & 1 else 1
                g0, g1 = 2 * pr, 2 * pr + 1
                s8[p, pr, v, 0, g0 * O_PER_GRP + o_loc] = w0
                s8[p, pr, v, 1, g1 * O_PER_GRP + o_loc] = w1
    # colsum selector: partition (jj, c) -> column c (c < 16)
    sel4 = np.zeros((128, O_PER_CORE), dtype=BF16)
    for jj in range(JQ):
        for c in range(O_PER_CORE):
            sel4[MW * jj + c, c] = 1
    ident = np.eye(128, dtype=BF16)

    # host-side P[i, o] = sum_k m[i, o, k]; each o belongs to exactly one
    # group g = (o mod 16) // 4, so the relu correction for row (jj, o)
    # is P[i, o] masked by whether that region's group-g tile is
    # relu-produced (label != 'A').  Only consistency with the device's
    # bf16 m matters (the +P/-P copies cancel exactly on the diagonal).
    m_host = (x @ T.reshape(IN_F, OUT_F * KD)).reshape(B, OUT_F, KD)
    P_all = m_host.sum(axis=-1)                                 # [B, 128]

    in_maps = []
    for core in range(N_CORES):
        t_slice = T[:, core * O_PER_CORE:(core + 1) * O_PER_CORE, :]
        # [g][p(in_f within chunk), c, (4o,32k) col]
        tw = t_slice.reshape(IN_F // 128, 128, N_GRP, 128)
        ttg = tw.transpose(2, 1, 0, 3).astype(BF16)       # [4, 128, 4, 128]
        txg = np.concatenate([ttg[0], xt], axis=2)        # [128, 4, 256]
        txg = np.ascontiguousarray(txg)
        tt = np.ascontiguousarray(ttg[1:])
        P = P_all[:, core * O_PER_CORE:(core + 1) * O_PER_CORE]  # [B, 16]
        Pb = P.astype(BF16)                   # bf16-quantized, used as-is
        Pf = Pb.astype(np.float32)
        # c1[i, q, row(jj,c)] = -bf16(P_S); bq[row, q] = -f32(bf16(P_S))[j]
        c1 = np.zeros((B, N_QUAD, 128), dtype=BF16)
        bq = np.zeros((128, N_QUAD), dtype=np.float32)
        for q in range(N_QUAD):
            for jj in range(JQ):
                labels = _REG[q][jj]
                mask = np.array([labels[c // O_PER_GRP] != "A"
                                 for c in range(O_PER_CORE)])
                c1[:, q, MW * jj:MW * jj + O_PER_CORE] = \
                    np.where(mask[None, :], -Pf, 0.0).astype(BF16)
                bq[MW * jj:MW * jj + O_PER_CORE, q] = \
                    np.where(mask, -Pf[JQ * q + jj, :], 0.0)
        in_maps.append({"tt": tt, "txg": txg, "s2b": s2b, "s8": s8,
                        "sel4": sel4, "idm": ident, "c1": c1, "bq": bq})

    res = run_bass_kernel_spmd(nc, in_maps, core_ids=list(range(N_CORES)))

    ob_full = np.empty((B, OUT_F), dtype=np.float32)
    for c, r in enumerate(res.results):
        rs = r["rs"]                                            # [128, 32]
        ac = r["accs"]                                          # [16, 124]
        row = rs.reshape(JQ, MW, N_QUAD)[:, :O_PER_CORE, :]     # [jj, r, q]
        ob = row.transpose(2, 0, 1).reshape(B, O_PER_CORE)      # [j, r]
        ob[JQ:, :] += ac.T                                      # j >= 4
        ob_full[:, c * O_PER_CORE:(c + 1) * O_PER_CORE] = ob
    out = np.concatenate([x, ob_full - 1.0], axis=1).astype(np.float32)
    return out


if __name__ == "__main__":
    print("plan loads (ns):", {k: round(v) for k, v in _LOAD.items()})
    n8 = sum(p[0] + p[1] for p in _PAIR8)
    print(f"fp8 DR pairs: {n8}/64")
